# revision 37
# baseline (speedup 1.0000x reference)
"""MHSA Trainium2 Bass kernel (v2).

Problem: B=4, P=4096, C=256, H=4 heads, D=64, fp32.
  q/k/v = x @ W{q,k,v} + b;  att = softmax(q k^T / sqrt(D)); out = (att v) @ Wo + bo

Sharding: 8 cores = (batch b, sequence half), SPMD-uniform, no collectives.
Each core computes the full attention output for 2048 query rows of one
batch; K/V come from the full 4096-row x of that batch (second half gets x
rolled by -2048 rows; softmax over keys is permutation invariant).

v2 design (cost-model driven):
  - Q/K projections write fp8(e4m3) Q^T/K^T in a d-split layout
    [32 parts @ 32h, 2 planes, seq] so the S^T = K^T q matmuls run in fp8
    DoubleRow perf mode (2x PE throughput, full d=64 contraction in one
    instruction per (ktile, head)). The d-split needs no on-chip shuffling:
    Wq/Wk columns are permuted on the host (and scaled by 8 to keep fp8
    quantization away from the subnormal range; the exp scale absorbs the
    64x).
  - exp(S) tiles are column-split between the ACT engine (hardware Exp)
    and a custom DVE op EXP_P4_ANT computing ((c3 x + c2) x + c1) x + 1
    squared twice -- a degree-3 fit of exp(x/4), ~7e-5 max rel error over
    the logit range (|S*scale| <= 0.9), fitted at build time. Both write
    fp16 P tiles. Splitting every tile keeps per-tile exp latency under
    the PE's per-ktile time so the 2-buffer S^T PSUM never backpressures.
  - PV runs fp16 (V fp16 with a ones column accumulating the softmax
    denominator as PSUM row 64), 1 cycle/row.
  - V/Wo biases are added with rank-1 matmuls (ones x bias row); Q/K
    biases ride the Identity/tensor_scalar psum->sbuf copies.
  - PV matmuls trail S^T through a software FIFO (one unit per ktile,
    gentle catch-up), so the in-order PE queue never bunches them. The
    previous head pair's lagged PVs drain inside the next pair's loop,
    then its normalize chain (wide reciprocal on the PSUM denominator row
    -> partition_broadcast on the idle Pool engine -> per-head
    normalize-mul on DVE) is emitted hook-by-hook before the new pair's
    first PV may overwrite the single PSUM O buffer. The Wo projection
    runs as a phase 3 when the attention PSUM pools are free again.
"""

import numpy as np

B, P, C, H, D = 4, 4096, 256, 4, 64
PQ = P // 2          # query rows per core
NPT = P // 128       # 32 key tiles
SCALE = float(D) ** -0.5
QK_PRESCALE = 8.0    # q,k each scaled by 8 on host (folded into Wq/Wk/bq/bk)
EXP_SCALE = SCALE / (QK_PRESCALE * QK_PRESCALE)
N_CORES = 8
POLY_RNG = 0.9       # |S*SCALE| fit range for the DVE exp polynomial
ACT_COLS = 298       # exp columns (of 512) on ACT; rest on DVE
PV_BUFS = 27         # p-tile pool depth (PV pops trail S^T via a FIFO)

_CACHE = {}
DEBUG_DUMPS = False


# ---------- custom DVE exp op ----------

def _register_exp_op():
    from concourse.dve_spec import Spec, Src0, C0, C1, C2, One, sq, lower
    from concourse.dve_ops import (
        OPS,
        CUSTOM_DVE_SPECS,
        DveOp,
        DveOpSpec,
        _CUSTOM_DVE_ROW_BASE,
        _SUB_OPCODE_FOR_NAME,
    )

    name = "EXP_P4_ANT"
    if name in _SUB_OPCODE_FOR_NAME:
        return next(o for o in OPS if o.name == name)
    p = ((Src0 * C0 + C1) * Src0 + C2) * Src0 + One
    spec = Spec(
        body=sq(sq(p)),
        reference=lambda in0, in1, s0, s1, imm2: (
            ((((in0.astype(np.float32) * s0 + s1) * in0 + imm2) * in0 + 1.0) ** 2)
            ** 2
        ),
    )
    row = _CUSTOM_DVE_ROW_BASE + len(OPS)
    shas = {}
    for ver in ("v3", "v4"):
        s = DveOpSpec(name=name, opcode=row, uops=lower(spec, ver=ver), rd1_en=False)
        shas[ver] = s.sha(ver)
    op = DveOp(name, spec, subdim=False, uops_sha=shas)
    _SUB_OPCODE_FOR_NAME[name] = row
    OPS.append(op)
    CUSTOM_DVE_SPECS[name] = spec
    return op


def _exp_poly_coeffs(s):
    """Minimax-ish fit: exp(x*s) ~ (((c3 x + c2) x + c1) x + 1)^4, |x*s|<=POLY_RNG."""
    y = np.linspace(-POLY_RNG, POLY_RNG, 20001)
    t = np.exp(y / 4.0)
    A = np.stack([y, y**2, y**3], axis=1)
    b = t - 1.0
    w = np.ones_like(y)
    for _ in range(60):
        c = np.linalg.lstsq(A * w[:, None], b * w, rcond=None)[0]
        p = 1.0 + A @ c
        rel = np.abs(p**4 - np.exp(y)) / np.exp(y)
        w = w * (1e-12 + rel) ** 0.5
        w /= w.max()
    c1, c2, c3 = c
    return float(c3 * s**3), float(c2 * s**2), float(c1 * s)  # s0, s1, imm2


def _build():
    from contextlib import ExitStack

    import concourse.bass as bass
    import concourse.mybir as mybir
    import concourse.tile as tile
    from concourse import bacc
    from concourse.masks import make_identity

    EXP_OP = _register_exp_op()
    EC0, EC1, EC2 = _exp_poly_coeffs(EXP_SCALE)

    F32 = mybir.dt.float32
    F16 = mybir.dt.float16
    F32R = mybir.dt.float32r
    F8 = mybir.dt.float8e4
    EXP = mybir.ActivationFunctionType.Exp
    IDENT = mybir.ActivationFunctionType.Identity
    DR = mybir.MatmulPerfMode.DoubleRow

    nc = bacc.Bacc("TRN2", target_bir_lowering=False, debug=False)

    x_d = nc.dram_tensor("x", [P, C], F32, kind="ExternalInput")
    w_d = {
        nm: nc.dram_tensor(nm, [C, C], F32, kind="ExternalInput")
        for nm in ("Wq", "Wk", "Wv", "Wo")
    }
    b_d = {
        nm: nc.dram_tensor(nm, [C], F32, kind="ExternalInput")
        for nm in ("bq", "bk", "bv", "bo")
    }
    out_d = nc.dram_tensor("out", [PQ, C], F32, kind="ExternalOutput")
    dbg = {}
    if DEBUG_DUMPS:
        dbg["xT"] = nc.dram_tensor("dbg_xT", [128, 2, P], F16, kind="ExternalOutput")
        dbg["KT8"] = nc.dram_tensor("dbg_KT8", [128, 2, P], F8, kind="ExternalOutput")
        dbg["QT8"] = nc.dram_tensor("dbg_QT8", [128, 2, PQ], F8, kind="ExternalOutput")
        dbg["Vp"] = nc.dram_tensor(
            "dbg_Vp", [128, NPT, H, D + 1], F16, kind="ExternalOutput"
        )
        dbg["OT"] = nc.dram_tensor("dbg_OT", [128, 2, PQ], F16, kind="ExternalOutput")
        dbg["p0"] = nc.dram_tensor("dbg_p0", [128, 2, 512], F16, kind="ExternalOutput")
        dbg["osb0"] = nc.dram_tensor(
            "dbg_osb0", [D + 1, 2, 512], F32, kind="ExternalOutput"
        )

    with tile.TileContext(nc) as tc, ExitStack() as ctx:
        const = ctx.enter_context(tc.tile_pool(name="const", bufs=1))
        big = ctx.enter_context(tc.tile_pool(name="big", bufs=1))
        stage = ctx.enter_context(tc.tile_pool(name="stage", bufs=2))
        ptiles = ctx.enter_context(tc.tile_pool(name="ptiles", bufs=PV_BUFS))
        small = ctx.enter_context(tc.tile_pool(name="small", bufs=4))
        outp = ctx.enter_context(tc.tile_pool(name="outp", bufs=2))

        # ---- x DMA first (biggest transfer, heads the HWDGE queue) ----
        xs_tiles = []
        for tq in range(8):
            xs = stage.tile([128, 4, C], F32, tag="xin", name=f"xin{tq}")
            if tq == 0:
                # split the first chunk so the transposes start sooner
                for half in range(2):
                    nc.sync.dma_start(
                        out=xs[:, 2 * half : 2 * half + 2, :],
                        in_=x_d[half * 256 : (half + 1) * 256, :].rearrange(
                            "(t p) c -> p t c", t=2
                        ),
                    )
            else:
                nc.sync.dma_start(
                    out=xs,
                    in_=x_d[tq * 512 : (tq + 1) * 512, :].rearrange(
                        "(t p) c -> p t c", t=4
                    ),
                )
            xs_tiles.append(xs)

        ident = const.tile([128, 128], F32, tag="ident")
        make_identity(nc, ident)
        ones128 = const.tile([1, 128], F16, tag="ones128")
        nc.gpsimd.memset(ones128[:], 1.0)

        # ---- weights: DMA f32, convert to fp16 on Pool ----
        w16 = {}
        for nm in ("Wq", "Wk", "Wv", "Wo"):
            wst = stage.tile([128, 2, C], F32, tag=f"wst_{nm}", name=f"wst_{nm}")
            nc.sync.dma_start(
                out=wst, in_=w_d[nm][:, :].rearrange("(c p) d -> p c d", p=128)
            )
            t = const.tile([128, 2, C], F16, tag=f"w16_{nm}")
            nc.gpsimd.tensor_copy(out=t, in_=wst)
            w16[nm] = t

        # channel-major per-partition biases for the Q/K psum->sbuf copies
        bias_qk = {}
        for nm in ("bq", "bk"):
            t = const.tile([128, 2], F32, tag=f"b_{nm}")
            nc.sync.dma_start(out=t, in_=b_d[nm][:].rearrange("(c p) -> p c", p=128))
            bias_qk[nm] = t
        # fp16 bias rows for the rank-1 bias matmuls (V, out)
        bias_row = {}
        for nm in ("bv", "bo"):
            st = stage.tile([1, C], F32, tag=f"brs_{nm}", name=f"brs_{nm}")
            nc.sync.dma_start(out=st, in_=b_d[nm][None, :])
            t = const.tile([1, C], F16, tag=f"br_{nm}")
            nc.gpsimd.tensor_copy(out=t, in_=st)
            bias_row[nm] = t

        xT = big.tile([128, 2, P], F16, tag="xT")
        KT8 = big.tile([128, 2, P], F8, tag="KT8")
        QT8 = big.tile([128, 2, PQ], F8, tag="QT8")
        Vp = big.tile([128, NPT, H, D + 1], F16, tag="Vp")
        OT = big.tile([128, 2, PQ], F16, tag="OT")

        # softmax-denominator ones column of V
        nc.gpsimd.memset(Vp[:, :, :, D : D + 1], 1.0)

        # ---- phase 1: x^T (fp16), Q^T/K^T (fp8 d-split), V (fp16) ----
        with (
            tc.tile_pool(name="ps_tr", bufs=3, space="PSUM") as ps_tr,
            tc.tile_pool(name="ps_pj", bufs=3, space="PSUM") as ps_pj,
            tc.tile_pool(name="ps_pv", bufs=2, space="PSUM") as ps_pv,
        ):
            flip = 0
            for tq in range(8):
                for t2 in range(2):
                    # 2 x-tiles -> one [128,4,128] psum group of f32r transposes
                    tp = ps_tr.tile([128, 4, 128], F32, tag="tr")
                    for ti in range(2):
                        for c2 in range(2):
                            nc.tensor.transpose(
                                tp[:, 2 * ti + c2, :],
                                xs_tiles[tq][
                                    :, 2 * t2 + ti, c2 * 128 : (c2 + 1) * 128
                                ],
                                ident,
                            )
                    base = tq * 512 + t2 * 256
                    # psum order (t', c2, p) -> xT[:, c2, base + 128 t' + p]
                    dst = xT[:, :, base : base + 256].rearrange(
                        "p c (t k) -> p t c k", t=2
                    )
                    srcap = tp.rearrange("p (t c) k -> p t c k", t=2)
                    if flip % 2 == 0:
                        nc.scalar.copy(out=dst, in_=srcap)
                    else:
                        nc.vector.tensor_copy(out=dst, in_=srcap)
                    flip += 1

            # Q^T/K^T projections -> fp8 d-split (host-permuted weights).
            # mt-outer so attention can consume chunk g as soon as its two
            # planes are written.
            for dst8, w, bias, nmt in (
                (QT8, w16["Wq"], bias_qk["bq"], PQ // 512),
                (KT8, w16["Wk"], bias_qk["bk"], P // 512),
            ):
                for mt in range(nmt):
                    for c2 in range(2):
                        pp = ps_pj.tile([128, 512], F32, tag="proj")
                        for ci in range(2):
                            nc.tensor.matmul(
                                pp,
                                lhsT=w[:, ci, c2 * 128 : (c2 + 1) * 128],
                                rhs=xT[:, ci, mt * 512 : (mt + 1) * 512],
                                start=(ci == 0),
                                stop=(ci == 1),
                            )
                        dstap = dst8[:, c2, mt * 512 : (mt + 1) * 512]
                        if flip % 2 == 0:
                            nc.scalar.activation(
                                out=dstap, in_=pp, func=IDENT, bias=bias[:, c2 : c2 + 1]
                            )
                        else:
                            nc.vector.tensor_scalar_add(
                                out=dstap, in0=pp, scalar1=bias[:, c2 : c2 + 1]
                            )
                        flip += 1

            # V projection (fp16) + rank-1 bias matmul
            for pt in range(NPT):
                pv = ps_pv.tile([128, C], F32, tag="vproj")
                for ci in range(2):
                    nc.tensor.matmul(
                        pv,
                        lhsT=xT[:, ci, pt * 128 : (pt + 1) * 128],
                        rhs=w16["Wv"][:, ci, :],
                        start=(ci == 0),
                        stop=False,
                    )
                nc.tensor.matmul(
                    pv, lhsT=ones128, rhs=bias_row["bv"], start=False, stop=True
                )
                dstap = Vp[:, pt, :, 0:D]
                srcap = pv.rearrange("p (h d) -> p h d", h=H)
                if flip % 2 == 0:
                    nc.scalar.copy(out=dstap, in_=srcap)
                else:
                    nc.vector.tensor_copy(out=dstap, in_=srcap)
                flip += 1

        # ---- phase 2: attention + output projection ----
        with (
            tc.tile_pool(name="ps_s", bufs=3, space="PSUM") as ps_s,
            tc.tile_pool(name="ps_o", bufs=1, space="PSUM") as ps_o,
        ):
            pending = []  # deferred hooks, emitted inside the NEXT pair's loop

            def make_normalize(o_ps, heads, m, pair):
                """One wide recip+partition_broadcast for both heads, then a
                normalize-mul per head (DVE, one PSUM operand) -> OT fp16."""
                st = {}

                def bcast_both():
                    rc = small.tile([1, 2, 512], F32, tag="recip")
                    with nc.allow_low_precision(reason="f32 recip ~1e-5"):
                        nc.vector.reciprocal(out=rc, in_=o_ps[D : D + 1, :, :])
                    bcs = small.tile([64, 2, 512], F32, tag="rbcast")
                    nc.gpsimd.partition_broadcast(bcs, rc)
                    st["bcs"] = bcs

                def mul_head(j, h):
                    def fn():
                        bp, ch = 64 * (h % 2), h // 2
                        nc.vector.tensor_mul(
                            out=OT[bp : bp + 64, ch, m * 512 : (m + 1) * 512],
                            in0=o_ps[0:D, j, :],
                            in1=st["bcs"][:, j, :],
                        )
                    return fn

                return [bcast_both] + [mul_head(j, h) for j, h in enumerate(heads)]

            pv_fifo = []  # pending PV closures, popped one per ktile step
            for m in range(PQ // 512):
                for pair in range(2):
                    heads = (2 * pair, 2 * pair + 1)
                    o_ps = ps_o.tile([D + 1, 2, 512], F32, tag="o")
                    p_tiles = [None] * NPT
                    hooks, pending = pending, []

                    def issue_pv(g, o_ps=o_ps, p_tiles=p_tiles, heads=heads):
                        for j, h in enumerate(heads):
                            nc.tensor.matmul(
                                o_ps[:, j, :],
                                lhsT=Vp[:, g, h, :],
                                rhs=p_tiles[g][:, j, :],
                                start=(g == 0),
                                stop=(g == NPT - 1),
                                skip_group_check=True,
                            )

                    prev_remaining = len(pv_fifo)
                    hook_i = 0
                    last_hook_g = -2
                    for g in range(NPT):
                        s_ps = ps_s.tile([128, 2, 512], F32, tag="s")
                        for j, h in enumerate(heads):
                            nc.tensor.matmul(
                                s_ps[:, j, :],
                                lhsT=KT8[
                                    32 * h : 32 * h + 32, :, g * 128 : (g + 1) * 128
                                ],
                                rhs=QT8[
                                    32 * h : 32 * h + 32, :, m * 512 : (m + 1) * 512
                                ],
                                start=True,
                                stop=True,
                                perf_mode=DR,
                                tile_position=(32 * h, 0),
                            )
                        p_sb = ptiles.tile([128, 2, 512], F16, tag="p", name="p")
                        p_tiles[g] = p_sb
                        # column-split exp: ACT | DVE in parallel on every tile
                        nc.scalar.activation(
                            out=p_sb[:, :, 0:ACT_COLS],
                            in_=s_ps[:, :, 0:ACT_COLS],
                            func=EXP,
                            scale=EXP_SCALE,
                        )
                        nc.vector._custom_dve(
                            EXP_OP,
                            out=p_sb[:, :, ACT_COLS:512],
                            in0=s_ps[:, :, ACT_COLS:512],
                            s0=EC0,
                            s1=EC1,
                            imm2=EC2,
                        )
                        if DEBUG_DUMPS and m == 0 and pair == 0 and g == 0:
                            nc.sync.dma_start(out=dbg["p0"][:, :, :], in_=p_sb)
                        if g >= 3:
                            pv_fifo.append(lambda g=g - 3, f=issue_pv: f(g))
                        # normalize hooks for the PREVIOUS pair: only after all
                        # its PV units drained (they write the o_ps the hooks
                        # read); own-pair PV pops only after all hooks emitted
                        # (they overwrite that same single o_ps buffer).
                        if (
                            hook_i < len(hooks)
                            and prev_remaining == 0
                            and g - last_hook_g >= 2
                        ):
                            hooks[hook_i]()
                            hook_i += 1
                            last_hook_g = g
                        if pv_fifo:
                            if prev_remaining > 0:
                                n = 2 if (len(pv_fifo) > 6 and g % 2 == 0) else 1
                                for _ in range(min(n, prev_remaining)):
                                    pv_fifo.pop(0)()
                                    prev_remaining -= 1
                            elif hook_i == len(hooks):
                                pv_fifo.pop(0)()
                                if len(pv_fifo) > 6 and g % 2 == 0:
                                    pv_fifo.pop(0)()
                    for g in range(NPT - 3, NPT):
                        pv_fifo.append(lambda g=g, f=issue_pv: f(g))

                    pending = make_normalize(o_ps, heads, m, pair)

            # drain the last pair's lagged PVs + normalize
            for fn in pv_fifo:
                fn()
            for fn in pending:
                fn()

        # ---- phase 3: output projection (PSUM banks free again) ----
        with tc.tile_pool(name="ps_w", bufs=4, space="PSUM") as ps_w:
            for m in range(PQ // 512):
                ot_sb = outp.tile([128, 4, C], F32, tag="osb", name=f"osb{m}")
                for qt in range(4):
                    pi = m * 4 + qt
                    wp = ps_w.tile([128, C], F32, tag="wo")
                    for ci in range(2):
                        nc.tensor.matmul(
                            wp,
                            lhsT=OT[:, ci, pi * 128 : (pi + 1) * 128],
                            rhs=w16["Wo"][:, ci, :],
                            start=(ci == 0),
                            stop=False,
                        )
                    nc.tensor.matmul(
                        wp, lhsT=ones128, rhs=bias_row["bo"], start=False, stop=True
                    )
                    if qt % 2 == 0:
                        nc.scalar.copy(out=ot_sb[:, qt, :], in_=wp)
                    else:
                        nc.vector.tensor_copy(out=ot_sb[:, qt, :], in_=wp)
                    nc.sync.dma_start(
                        out=out_d[pi * 128 : (pi + 1) * 128, :],
                        in_=ot_sb[:, qt, :],
                    )

        if DEBUG_DUMPS:
            nc.sync.dma_start(out=dbg["OT"][:, :, :], in_=OT)

    nc.compile()
    return nc


def _get_nc():
    if "nc" not in _CACHE:
        _CACHE["nc"] = _build()
    return _CACHE["nc"]


def _qk_perm():
    """Column permutation for Wq/Wk: output channel c=64h+d -> fp8 d-split
    position 128*(d//32) + 32*h + (d%32)."""
    perm = np.zeros(C, dtype=np.int64)
    for h in range(H):
        for d in range(D):
            perm[128 * (d // 32) + 32 * h + (d % 32)] = 64 * h + d
    return perm


def _in_maps(inputs):
    x = np.ascontiguousarray(np.asarray(inputs["x"], dtype=np.float32))
    assert x.shape == (B, P, C), x.shape
    perm = _qk_perm()
    shared = {}
    for nm in ("Wv", "Wo", "bv", "bo"):
        shared[nm] = np.ascontiguousarray(np.asarray(inputs[nm], dtype=np.float32))
    for nm in ("Wq", "Wk"):
        w = np.asarray(inputs[nm], dtype=np.float32) * QK_PRESCALE
        shared[nm] = np.ascontiguousarray(w[:, perm])
    for nm in ("bq", "bk"):
        b = np.asarray(inputs[nm], dtype=np.float32) * QK_PRESCALE
        shared[nm] = np.ascontiguousarray(b[perm])
    maps = []
    for core in range(N_CORES):
        b, half = core // 2, core % 2
        if half == 0:
            xl = np.ascontiguousarray(x[b])
        else:
            xl = np.ascontiguousarray(np.roll(x[b], -PQ, axis=0))
        maps.append({"x": xl, **shared})
    return maps


def run(inputs, trace=False):
    from concourse import bass_utils

    nc = _get_nc()
    res = bass_utils.run_bass_kernel_spmd(
        nc, _in_maps(inputs), core_ids=list(range(N_CORES)), trace=trace
    )
    out = np.empty((B, P, C), np.float32)
    for core in range(N_CORES):
        b, half = core // 2, core % 2
        out[b, half * PQ : (half + 1) * PQ] = res.results[core]["out"]
    return out, res


def kernel(**inputs):
    out, _ = run(inputs, trace=False)
    return out
